# revision 15
# baseline (speedup 1.0000x reference)
"""LogLinearAttention TRN2 kernel: 8-core data-parallel over (batch, token-half).

Each core handles 2048 tokens (4 chunks of 512) of one batch element:
  core i -> batch i//2, tokens (i%2)*2048 ... +2048.
Block-local attention makes (batch, chunk) units fully independent, so no
collectives are needed; the host shards inputs and concatenates outputs.

Layout strategy (all matmuls out = lhsT.T @ rhs, fp32r for full PE rate):
  - qkv projection emits qT,kT as [feat, tok] (weights stationary); v emitted
    as [tok, feat] via the transposed orientation (x stationary).
  - scores computed as S.T [key, q]; key gate lam applied per-partition as the
    activation scale of a fused exp (no max-subtraction: |scores|<=10).
  - AV uses a ones-augmented V (65th column) so the softmax denominator comes
    out as row 64 of the same accumulation.
  - normalization defers to a selector-matmul broadcast of 1/sum, applied to
    attn_out.T, which is exactly the lhsT/rhs layout the output projection
    needs. Output is written transposed; host transposes back.
"""
import sys
sys.path.insert(0, "/opt/trn_rl_repo")
import numpy as np
import concourse.bass as bass
import concourse.mybir as mybir
from concourse.tile import TileContext
from concourse.bass_utils import run_bass_kernel_spmd

B, T, C = 4, 4096, 1024
H, NCH = 16, 8
DH = C // H          # 64
CS = T // NCH        # 512 chunk size
TPC = T // 2         # tokens per core (2048)
NCHC = TPC // CS     # chunks per core (4)
F32R = mybir.dt.float32r
F32 = mybir.dt.float32


def split_multi_waits(nc):
    """This walrus build allows one sync wait per instruction; hoist extras
    into NoOps on the same engine queue."""
    for f in nc.m.functions:
        for blk in f.blocks:
            new_insts = []
            for inst in blk.instructions:
                si = inst.sync_info
                if si is not None and si.on_wait is not None and len(si.on_wait) > 1:
                    waits = list(si.on_wait)
                    for j, w in enumerate(waits[:-1]):
                        nop = mybir.InstNoOp(
                            name=f"{inst.name}-ws{j}", engine=inst.engine, ins=[], outs=[]
                        )
                        nop.sync_info = mybir.SyncInfo(on_wait=[w], on_update=[])
                        new_insts.append(nop)
                    inst.sync_info = mybir.SyncInfo(
                        on_wait=[waits[-1]], on_update=list(si.on_update or [])
                    )
                new_insts.append(inst)
            blk.instructions = new_insts


def build_program():
    nc = bass.Bass(target_bir_lowering=False, trn_type="TRN2")
    AF = mybir.ActivationFunctionType

    xT_d = nc.dram_tensor("xT", [C, TPC], F32R, kind="ExternalInput")
    wqkvT_d = nc.dram_tensor("wqkvT", [C, 3 * C], F32R, kind="ExternalInput")
    woutT_d = nc.dram_tensor("woutT", [C, C], F32R, kind="ExternalInput")
    bout_d = nc.dram_tensor("bout2d", [128, 8], F32, kind="ExternalInput")
    l1_d = nc.dram_tensor("l1_lhsT", [2, 64], F32R, kind="ExternalInput")
    logpos_d = nc.dram_tensor("logpos_aug", [2, TPC], F32R, kind="ExternalInput")
    l2_d = nc.dram_tensor("l2_rhs", [65, 16], F32R, kind="ExternalInput")
    sel_d = nc.dram_tensor("sel", [16, C], F32R, kind="ExternalInput")
    outT_d = nc.dram_tensor("outT", [C, TPC], F32, kind="ExternalOutput")

    with TileContext(nc) as tc, nc.allow_low_precision(reason="fp32r operands round to tf32-class; accumulation stays fp32"):
        with tc.tile_pool(name="const", bufs=1) as cst, \
             tc.tile_pool(name="wout", bufs=1) as wpool, \
             tc.tile_pool(name="ps", bufs=6, space="PSUM") as ps:

            # ---- resident constants ----
            wout_sb = []
            for kt in range(8):
                w = wpool.tile([128, C], F32R, name=f"wout{kt}", tag=f"wout{kt}")
                nc.sync.dma_start(out=w[:], in_=woutT_d.ap()[kt * 128:(kt + 1) * 128, :])
                wout_sb.append(w)
            bout_sb = cst.tile([128, 8], F32, tag="bout")
            nc.sync.dma_start(out=bout_sb[:], in_=bout_d.ap())
            sel_sb = cst.tile([16, C], F32R, tag="sel")
            nc.sync.dma_start(out=sel_sb[:], in_=sel_d.ap())
            l1_sb = cst.tile([2, 64], F32R, tag="l1")
            nc.sync.dma_start(out=l1_sb[:], in_=l1_d.ap())
            l2_sb = cst.tile([65, 16], F32R, tag="l2")
            nc.sync.dma_start(out=l2_sb[:], in_=l2_d.ap())

            # ---- lambda gate MLP: lam_sb[key_part, tt*16+h] ----
            lam_sb = cst.tile([128, 16 * 16], F32, tag="lam")
            with tc.tile_pool(name="pre", bufs=1) as pre:
                h1_sb = pre.tile([65, TPC], F32R, tag="h1")
                lp = pre.tile([2, TPC], F32R, tag="lp")
                nc.sync.dma_start(out=lp[:], in_=logpos_d.ap())
                nc.sync.dma_start(out=h1_sb[64:65, :], in_=logpos_d.ap()[1:2, :])
                for j in range(4):
                    ph = ps.tile([64, 512], F32, name=f"ph{j}", tag="b512")
                    nc.tensor.matmul(ph[:], l1_sb[:], lp[:, j * 512:(j + 1) * 512], start=True, stop=True)
                    nc.scalar.activation(h1_sb[0:64, j * 512:(j + 1) * 512], ph[:], AF.Relu)
                for tt in range(16):
                    pl = ps.tile([128, 16], F32, name=f"pl{tt}", tag="b16", bufs=2)
                    nc.tensor.matmul(pl[:], h1_sb[:, tt * 128:(tt + 1) * 128], l2_sb[:],
                                     start=True, stop=True)
                    nc.scalar.activation(lam_sb[:, tt * 16:(tt + 1) * 16], pl[:], AF.Sigmoid)

            # ---- main loop over chunks ----
            ctx2 = [tc.tile_pool(name="xs", bufs=8), tc.tile_pool(name="wblk", bufs=4),
                    tc.tile_pool(name="qk", bufs=16), tc.tile_pool(name="vv", bufs=4),
                    tc.tile_pool(name="pt", bufs=5), tc.tile_pool(name="at", bufs=8),
                    tc.tile_pool(name="atn", bufs=8), tc.tile_pool(name="ob", bufs=2)]
            xs = ctx2[0].__enter__(); wb = ctx2[1].__enter__(); qk = ctx2[2].__enter__()
            vv = ctx2[3].__enter__(); ptp = ctx2[4].__enter__(); at = ctx2[5].__enter__()
            atn = ctx2[6].__enter__(); ob = ctx2[7].__enter__()
            for c in range(NCHC):
                # x tiles for this chunk: xT rows = features, cols = tokens
                x_sb = []
                for kt in range(8):
                    xt = xs.tile([128, 512], F32R, name=f"x_c{c}_k{kt}", tag="x")
                    nc.sync.dma_start(
                        out=xt[:],
                        in_=xT_d.ap()[kt * 128:(kt + 1) * 128, c * 512:(c + 1) * 512])
                    x_sb.append(xt)

                # q,k projection: out.T tiles [feat 128, tok 512], feats 0..2047
                qkT = []
                for g in range(4):
                    for ft in range(4):
                        wt = wb.tile([128, 1024], F32R, name=f"w_c{c}_g{g}_f{ft}", tag="w")
                        nc.sync.dma_start(
                            out=wt.rearrange("p (kt m) -> p kt m", m=128),
                            in_=wqkvT_d.ap()[0:C, g * 512 + ft * 128:g * 512 + (ft + 1) * 128]
                                .rearrange("(kt p) m -> p kt m", p=128))
                        pq = ps.tile([128, 512], F32, name=f"pq_c{c}_{g}_{ft}", tag="b512")
                        for kt in range(8):
                            nc.tensor.matmul(pq[:], wt[:, kt * 128:(kt + 1) * 128],
                                             x_sb[kt][:], start=(kt == 0), stop=(kt == 7))
                        qt = qk.tile([128, 512], F32R, name=f"qkT_c{c}_{g * 4 + ft}", tag="qkT")
                        nc.scalar.copy(qt[:], pq[:])
                        qkT.append(qt)

                # v projection (x stationary): v_sb[tt] = [tok 128, 16*(64+1)]
                v_sb = []
                for tt in range(4):
                    vt = vv.tile([128, 16 * 65], F32R, name=f"v_c{c}_{tt}", tag="v")
                    v3 = vt.rearrange("p (h e) -> p h e", e=65)
                    nc.sync.dma_start(
                        out=v3[:, :, 64:65],
                        in_=logpos_d.ap()[1:2, 0:2048].rearrange("r (p e o) -> (r p) e o", p=128, o=1))
                    v_sb.append(vt)
                for g in range(2):
                    wvs = []
                    for kt in range(8):
                        wv = wb.tile([128, 512], F32R, name=f"wv_c{c}_{g}_{kt}", tag="wv", bufs=4)
                        nc.sync.dma_start(
                            out=wv[:],
                            in_=wqkvT_d.ap()[kt * 128:(kt + 1) * 128,
                                             2 * C + g * 512:2 * C + (g + 1) * 512])
                        wvs.append(wv)
                    for tt in range(4):
                        pv = ps.tile([128, 512], F32, name=f"pv_c{c}_{g}_{tt}", tag="b512")
                        for kt in range(8):
                            nc.tensor.matmul(pv[:], x_sb[kt][:, tt * 128:(tt + 1) * 128],
                                             wvs[kt][:],
                                             start=(kt == 0), stop=(kt == 7))
                        dst = v_sb[tt].rearrange("p (h e) -> p h e", e=65)[:, g * 8:(g + 1) * 8, 0:64]
                        src = pv.rearrange("p (h e) -> p h e", e=64)
                        nc.scalar.copy(dst, src)

                # attention per head
                sums_sb = at.tile([16, 512], F32, name=f"sums_c{c}", tag="sums")
                attnT = []
                for hp in range(8):
                    a_t = at.tile([128, 512], F32, name=f"attnT_c{c}_{hp}", tag="attnT")
                    attnT.append(a_t)
                for h in range(16):
                    hp, r0 = h // 2, (h % 2) * 64
                    kt_tile = qkT[8 + h // 2]
                    qt_tile = qkT[h // 2]
                    p_ts = []
                    for kk in range(4):
                        pscr = ps.tile([128, 512], F32, name=f"ps_c{c}_h{h}_{kk}", tag="b512")
                        nc.tensor.matmul(pscr[:],
                                         kt_tile[r0:r0 + 64, kk * 128:(kk + 1) * 128],
                                         qt_tile[r0:r0 + 64, :], start=True, stop=True)
                        p_t = ptp.tile([128, 512], F32R, name=f"p_c{c}_h{h}_{kk}", tag="p")
                        tt = c * 4 + kk
                        nc.scalar.activation(p_t[:], pscr[:], AF.Exp,
                                             scale=lam_sb[:, tt * 16 + h:tt * 16 + h + 1])
                        p_ts.append(p_t)
                    pav = ps.tile([128, 512], F32, name=f"pav_c{c}_h{h}", tag="b512")
                    for kk in range(4):
                        nc.tensor.matmul(pav[0:65, :],
                                         v_sb[kk][:, h * 65:(h + 1) * 65],
                                         p_ts[kk][:], start=(kk == 0), stop=(kk == 3))
                    nc.scalar.copy(attnT[hp][r0:r0 + 64, :], pav[0:64, :])
                    srow = at.tile([1, 512], F32, name=f"srow_c{c}_h{h}", tag="srow", bufs=2)
                    nc.scalar.copy(srow[:], pav[64:65, :])
                    nc.sync.dma_start(out=sums_sb[h:h + 1, :], in_=srow[:])

                # normalization via selector broadcast of 1/sums
                inv_sb = at.tile([16, 512], F32R, name=f"inv_c{c}", tag="inv")
                nc.vector.reciprocal(inv_sb[:], sums_sb[:])
                attnTn = []
                for hp in range(8):
                    pg = ps.tile([128, 512], F32, name=f"pg_c{c}_{hp}", tag="b512")
                    nc.tensor.matmul(pg[:], sel_sb[:, hp * 128:(hp + 1) * 128], inv_sb[:],
                                     start=True, stop=True)
                    an = atn.tile([128, 512], F32R, name=f"attnTn_c{c}_{hp}", tag="an")
                    nc.vector.tensor_mul(an[:], attnT[hp][:], pg[:])
                    attnTn.append(an)

                # output projection: final.T tiles [outC 128, tok 512]
                for of in range(8):
                    po = ps.tile([128, 512], F32, name=f"po_c{c}_{of}", tag="b512")
                    for kt in range(8):
                        nc.tensor.matmul(po[:], wout_sb[kt][:, of * 128:(of + 1) * 128],
                                         attnTn[kt][:], start=(kt == 0), stop=(kt == 7))
                    o_sb = ob.tile([128, 512], F32, name=f"o_c{c}_{of}", tag="o")
                    nc.scalar.activation(o_sb[:], po[:], AF.Identity,
                                         bias=bout_sb[:, of:of + 1])
                    nc.sync.dma_start(
                        out=outT_d.ap()[of * 128:(of + 1) * 128, c * 512:(c + 1) * 512],
                        in_=o_sb[:])
            for cm in reversed(ctx2):
                cm.__exit__(None, None, None)

    split_multi_waits(nc)
    return nc


_prog_cache = {}


def kernel(x, Wqkv, Wout, bout, Wl1, bl1, Wl2, bl2):
    x = np.ascontiguousarray(x, dtype=np.float32)
    scale = DH ** -0.5
    wqkvT = np.ascontiguousarray(Wqkv.T, dtype=np.float32)
    wqkvT[:, :C] *= scale  # fold attention scale into q projection
    woutT = np.ascontiguousarray(Wout.T, dtype=np.float32)
    bout2d = np.ascontiguousarray(bout.reshape(8, 128).T, dtype=np.float32)
    l1_lhsT = np.stack([Wl1[:, 0], bl1]).astype(np.float32)          # [2, 64]
    l2_rhs = np.concatenate([Wl2.T, bl2[None, :]], 0).astype(np.float32)  # [65, 16]
    sel = (np.arange(C)[None, :] // DH == np.arange(H)[:, None]).astype(np.float32)

    in_maps = []
    for core in range(8):
        b, half = core // 2, core % 2
        pos = half * TPC + np.arange(TPC, dtype=np.float32)
        logpos_aug = np.stack([np.log(pos + 1.0), np.ones(TPC, np.float32)]).astype(np.float32)
        xT = np.ascontiguousarray(x[b, half * TPC:(half + 1) * TPC, :].T)
        in_maps.append(dict(xT=xT, wqkvT=wqkvT, woutT=woutT, bout2d=bout2d,
                            l1_lhsT=l1_lhsT, logpos_aug=logpos_aug, l2_rhs=l2_rhs,
                            sel=sel))

    global _last_in_maps
    _last_in_maps = in_maps
    if "nc" not in _prog_cache:
        _prog_cache["nc"] = build_program()
    nc = _prog_cache["nc"]
    res = run_bass_kernel_spmd(nc, in_maps, core_ids=list(range(8)))

    out = np.empty((B, T, C), np.float32)
    for core in range(8):
        b, half = core // 2, core % 2
        out[b, half * TPC:(half + 1) * TPC, :] = res.results[core]["outT"].T
    return out


# revision 16
# speedup vs baseline: 1.6966x; 1.6966x over previous
"""LogLinearAttention TRN2 kernel: 8-core data-parallel over (batch, token-half).

Each core handles 2048 tokens (4 chunks of 512) of one batch element:
  core i -> batch i//2, tokens (i%2)*2048 ... +2048.
Block-local attention makes (batch, chunk) units fully independent, so no
collectives are needed; the host shards inputs and concatenates outputs.

Layout strategy (all matmuls out = lhsT.T @ rhs, fp32r for full PE rate):
  - qkv projection emits qT,kT as [feat, tok] (weights stationary); v emitted
    as [tok, feat] via the transposed orientation (x stationary).
  - scores computed as S.T [key, q]; key gate lam applied per-partition as the
    activation scale of a fused exp (no max-subtraction: |scores|<=10).
  - AV uses a ones-augmented V (65th column) so the softmax denominator comes
    out as row 64 of the same accumulation.
  - normalization defers to a selector-matmul broadcast of 1/sum, applied to
    attn_out.T, which is exactly the lhsT/rhs layout the output projection
    needs. Output is written transposed; host transposes back.
"""
import sys
sys.path.insert(0, "/opt/trn_rl_repo")
import numpy as np
import concourse.bass as bass
import concourse.mybir as mybir
from concourse.tile import TileContext
from concourse.bass_utils import run_bass_kernel_spmd

B, T, C = 4, 4096, 1024
H, NCH = 16, 8
DH = C // H          # 64
CS = T // NCH        # 512 chunk size
TPC = T // 2         # tokens per core (2048)
NCHC = TPC // CS     # chunks per core (4)
F32R = mybir.dt.float32r
F32 = mybir.dt.float32


def split_multi_waits(nc):
    """This walrus build allows one sync wait per instruction; hoist extras
    into NoOps on the same engine queue."""
    for f in nc.m.functions:
        for blk in f.blocks:
            new_insts = []
            for inst in blk.instructions:
                si = inst.sync_info
                if si is not None and si.on_wait is not None and len(si.on_wait) > 1:
                    waits = list(si.on_wait)
                    for j, w in enumerate(waits[:-1]):
                        nop = mybir.InstNoOp(
                            name=f"{inst.name}-ws{j}", engine=inst.engine, ins=[], outs=[]
                        )
                        nop.sync_info = mybir.SyncInfo(on_wait=[w], on_update=[])
                        new_insts.append(nop)
                    inst.sync_info = mybir.SyncInfo(
                        on_wait=[waits[-1]], on_update=list(si.on_update or [])
                    )
                new_insts.append(inst)
            blk.instructions = new_insts


def build_program():
    nc = bass.Bass(target_bir_lowering=False, trn_type="TRN2")
    AF = mybir.ActivationFunctionType

    xT_d = nc.dram_tensor("xT", [C, TPC], F32R, kind="ExternalInput")
    wqkvT_d = nc.dram_tensor("wqkvT", [C, 3 * C], F32R, kind="ExternalInput")
    woutT_d = nc.dram_tensor("woutT", [C, C], F32R, kind="ExternalInput")
    bout_d = nc.dram_tensor("bout2d", [128, 8], F32, kind="ExternalInput")
    l1_d = nc.dram_tensor("l1_lhsT", [2, 64], F32R, kind="ExternalInput")
    logpos_d = nc.dram_tensor("logpos_aug", [2, TPC], F32R, kind="ExternalInput")
    l2_d = nc.dram_tensor("l2_rhs", [65, 16], F32R, kind="ExternalInput")
    sel_d = nc.dram_tensor("sel", [16, C], F32R, kind="ExternalInput")
    outT_d = nc.dram_tensor("outT", [C, TPC], F32, kind="ExternalOutput")

    with TileContext(nc) as tc, nc.allow_low_precision(reason="fp32r operands round to tf32-class; accumulation stays fp32"):
        with tc.tile_pool(name="const", bufs=1) as cst, \
             tc.tile_pool(name="wout", bufs=1) as wpool, \
             tc.tile_pool(name="ps", bufs=8, space="PSUM") as ps:

            # ---- resident constants ----
            wout_sb = []
            for kt in range(8):
                w = wpool.tile([128, C], F32R, name=f"wout{kt}", tag=f"wout{kt}")
                nc.sync.dma_start(out=w[:], in_=woutT_d.ap()[kt * 128:(kt + 1) * 128, :])
                wout_sb.append(w)
            bout_sb = cst.tile([128, 8], F32, tag="bout")
            nc.sync.dma_start(out=bout_sb[:], in_=bout_d.ap())
            sel_sb = cst.tile([16, C], F32R, tag="sel")
            nc.sync.dma_start(out=sel_sb[:], in_=sel_d.ap())
            l1_sb = cst.tile([2, 64], F32R, tag="l1")
            nc.sync.dma_start(out=l1_sb[:], in_=l1_d.ap())
            l2_sb = cst.tile([65, 16], F32R, tag="l2")
            nc.sync.dma_start(out=l2_sb[:], in_=l2_d.ap())

            # ---- lambda gate MLP: lam_sb[key_part, tt*16+h] ----
            lam_sb = cst.tile([128, 16 * 16], F32, tag="lam")
            with tc.tile_pool(name="pre", bufs=1) as pre:
                h1_sb = pre.tile([65, TPC], F32R, tag="h1")
                lp = pre.tile([2, TPC], F32R, tag="lp")
                nc.sync.dma_start(out=lp[:], in_=logpos_d.ap())
                nc.sync.dma_start(out=h1_sb[64:65, :], in_=logpos_d.ap()[1:2, :])
                for j in range(4):
                    ph = ps.tile([64, 512], F32, name=f"ph{j}", tag="b512")
                    nc.tensor.matmul(ph[:], l1_sb[:], lp[:, j * 512:(j + 1) * 512], start=True, stop=True)
                    nc.scalar.activation(h1_sb[0:64, j * 512:(j + 1) * 512], ph[:], AF.Relu)
                for tt in range(16):
                    pl = ps.tile([128, 16], F32, name=f"pl{tt}", tag="b512")
                    nc.tensor.matmul(pl[:], h1_sb[:, tt * 128:(tt + 1) * 128], l2_sb[:],
                                     start=True, stop=True)
                    nc.scalar.activation(lam_sb[:, tt * 16:(tt + 1) * 16], pl[:], AF.Sigmoid)

            # ---- main loop over chunks ----
            ctx2 = [tc.tile_pool(name="xs", bufs=8), tc.tile_pool(name="wblk", bufs=4),
                    tc.tile_pool(name="qk", bufs=16), tc.tile_pool(name="vv", bufs=4),
                    tc.tile_pool(name="pt", bufs=5), tc.tile_pool(name="at", bufs=8),
                    tc.tile_pool(name="atn", bufs=8), tc.tile_pool(name="ob", bufs=2)]
            xs = ctx2[0].__enter__(); wb = ctx2[1].__enter__(); qk = ctx2[2].__enter__()
            vv = ctx2[3].__enter__(); ptp = ctx2[4].__enter__(); at = ctx2[5].__enter__()
            atn = ctx2[6].__enter__(); ob = ctx2[7].__enter__()
            for c in range(NCHC):
                # x tiles for this chunk: xT rows = features, cols = tokens
                x_sb = []
                for kt in range(8):
                    xt = xs.tile([128, 512], F32R, name=f"x_c{c}_k{kt}", tag="x")
                    nc.sync.dma_start(
                        out=xt[:],
                        in_=xT_d.ap()[kt * 128:(kt + 1) * 128, c * 512:(c + 1) * 512])
                    x_sb.append(xt)

                # q,k projection: out.T tiles [feat 128, tok 512], feats 0..2047
                qkT = []
                for g in range(4):
                    for ft in range(4):
                        wt = wb.tile([128, 1024], F32R, name=f"w_c{c}_g{g}_f{ft}", tag="w")
                        nc.sync.dma_start(
                            out=wt.rearrange("p (kt m) -> p kt m", m=128),
                            in_=wqkvT_d.ap()[0:C, g * 512 + ft * 128:g * 512 + (ft + 1) * 128]
                                .rearrange("(kt p) m -> p kt m", p=128))
                        pq = ps.tile([128, 512], F32, name=f"pq_c{c}_{g}_{ft}", tag="b512")
                        for kt in range(8):
                            nc.tensor.matmul(pq[:], wt[:, kt * 128:(kt + 1) * 128],
                                             x_sb[kt][:], start=(kt == 0), stop=(kt == 7))
                        qt = qk.tile([128, 512], F32R, name=f"qkT_c{c}_{g * 4 + ft}", tag="qkT")
                        nc.scalar.copy(qt[:], pq[:])
                        qkT.append(qt)

                # v projection (x stationary): v_sb[tt] = [tok 128, 16*(64+1)]
                v_sb = []
                for tt in range(4):
                    vt = vv.tile([128, 16 * 65], F32R, name=f"v_c{c}_{tt}", tag="v")
                    v3 = vt.rearrange("p (h e) -> p h e", e=65)
                    nc.sync.dma_start(
                        out=v3[:, :, 64:65],
                        in_=logpos_d.ap()[1:2, 0:2048].rearrange("r (p e o) -> (r p) e o", p=128, o=1))
                    v_sb.append(vt)
                for g in range(2):
                    wvs = []
                    for kt in range(8):
                        wv = wb.tile([128, 512], F32R, name=f"wv_c{c}_{g}_{kt}", tag="wv", bufs=4)
                        nc.sync.dma_start(
                            out=wv[:],
                            in_=wqkvT_d.ap()[kt * 128:(kt + 1) * 128,
                                             2 * C + g * 512:2 * C + (g + 1) * 512])
                        wvs.append(wv)
                    for tt in range(4):
                        pv = ps.tile([128, 512], F32, name=f"pv_c{c}_{g}_{tt}", tag="b512")
                        for kt in range(8):
                            nc.tensor.matmul(pv[:], x_sb[kt][:, tt * 128:(tt + 1) * 128],
                                             wvs[kt][:],
                                             start=(kt == 0), stop=(kt == 7))
                        dst = v_sb[tt].rearrange("p (h e) -> p h e", e=65)[:, g * 8:(g + 1) * 8, 0:64]
                        src = pv.rearrange("p (h e) -> p h e", e=64)
                        nc.scalar.copy(dst, src)

                # attention per head
                sums_sb = at.tile([16, 512], F32, name=f"sums_c{c}", tag="sums")
                attnT = []
                for hp in range(8):
                    a_t = at.tile([128, 512], F32, name=f"attnT_c{c}_{hp}", tag="attnT")
                    attnT.append(a_t)
                for h in range(16):
                    hp, r0 = h // 2, (h % 2) * 64
                    kt_tile = qkT[8 + h // 2]
                    qt_tile = qkT[h // 2]
                    p_ts = []
                    for kk in range(4):
                        pscr = ps.tile([128, 512], F32, name=f"ps_c{c}_h{h}_{kk}", tag="b512")
                        nc.tensor.matmul(pscr[:],
                                         kt_tile[r0:r0 + 64, kk * 128:(kk + 1) * 128],
                                         qt_tile[r0:r0 + 64, :], start=True, stop=True)
                        p_t = ptp.tile([128, 512], F32R, name=f"p_c{c}_h{h}_{kk}", tag="p")
                        tt = c * 4 + kk
                        nc.scalar.activation(p_t[:], pscr[:], AF.Exp,
                                             scale=lam_sb[:, tt * 16 + h:tt * 16 + h + 1])
                        p_ts.append(p_t)
                    pav = ps.tile([128, 512], F32, name=f"pav_c{c}_h{h}", tag="b512")
                    for kk in range(4):
                        nc.tensor.matmul(pav[0:65, :],
                                         v_sb[kk][:, h * 65:(h + 1) * 65],
                                         p_ts[kk][:], start=(kk == 0), stop=(kk == 3))
                    nc.scalar.copy(attnT[hp][r0:r0 + 64, :], pav[0:64, :])
                    srow = at.tile([1, 512], F32, name=f"srow_c{c}_h{h}", tag="srow", bufs=2)
                    nc.scalar.copy(srow[:], pav[64:65, :])
                    nc.sync.dma_start(out=sums_sb[h:h + 1, :], in_=srow[:])

                # normalization via selector broadcast of 1/sums
                inv_sb = at.tile([16, 512], F32R, name=f"inv_c{c}", tag="inv")
                nc.vector.reciprocal(inv_sb[:], sums_sb[:])
                attnTn = []
                for hp in range(8):
                    pg = ps.tile([128, 512], F32, name=f"pg_c{c}_{hp}", tag="b512")
                    nc.tensor.matmul(pg[:], sel_sb[:, hp * 128:(hp + 1) * 128], inv_sb[:],
                                     start=True, stop=True)
                    an = atn.tile([128, 512], F32R, name=f"attnTn_c{c}_{hp}", tag="an")
                    nc.vector.tensor_mul(an[:], attnT[hp][:], pg[:])
                    attnTn.append(an)

                # output projection: final.T tiles [outC 128, tok 512]
                for of in range(8):
                    po = ps.tile([128, 512], F32, name=f"po_c{c}_{of}", tag="b512")
                    for kt in range(8):
                        nc.tensor.matmul(po[:], wout_sb[kt][:, of * 128:(of + 1) * 128],
                                         attnTn[kt][:], start=(kt == 0), stop=(kt == 7))
                    o_sb = ob.tile([128, 512], F32, name=f"o_c{c}_{of}", tag="o")
                    nc.scalar.activation(o_sb[:], po[:], AF.Identity,
                                         bias=bout_sb[:, of:of + 1])
                    nc.sync.dma_start(
                        out=outT_d.ap()[of * 128:(of + 1) * 128, c * 512:(c + 1) * 512],
                        in_=o_sb[:])
            for cm in reversed(ctx2):
                cm.__exit__(None, None, None)

    split_multi_waits(nc)
    return nc


_prog_cache = {}


def kernel(x, Wqkv, Wout, bout, Wl1, bl1, Wl2, bl2):
    x = np.ascontiguousarray(x, dtype=np.float32)
    scale = DH ** -0.5
    wqkvT = np.ascontiguousarray(Wqkv.T, dtype=np.float32)
    wqkvT[:, :C] *= scale  # fold attention scale into q projection
    woutT = np.ascontiguousarray(Wout.T, dtype=np.float32)
    bout2d = np.ascontiguousarray(bout.reshape(8, 128).T, dtype=np.float32)
    l1_lhsT = np.stack([Wl1[:, 0], bl1]).astype(np.float32)          # [2, 64]
    l2_rhs = np.concatenate([Wl2.T, bl2[None, :]], 0).astype(np.float32)  # [65, 16]
    sel = (np.arange(C)[None, :] // DH == np.arange(H)[:, None]).astype(np.float32)

    in_maps = []
    for core in range(8):
        b, half = core // 2, core % 2
        pos = half * TPC + np.arange(TPC, dtype=np.float32)
        logpos_aug = np.stack([np.log(pos + 1.0), np.ones(TPC, np.float32)]).astype(np.float32)
        xT = np.ascontiguousarray(x[b, half * TPC:(half + 1) * TPC, :].T)
        in_maps.append(dict(xT=xT, wqkvT=wqkvT, woutT=woutT, bout2d=bout2d,
                            l1_lhsT=l1_lhsT, logpos_aug=logpos_aug, l2_rhs=l2_rhs,
                            sel=sel))

    global _last_in_maps
    _last_in_maps = in_maps
    if "nc" not in _prog_cache:
        _prog_cache["nc"] = build_program()
    nc = _prog_cache["nc"]
    res = run_bass_kernel_spmd(nc, in_maps, core_ids=list(range(8)))

    out = np.empty((B, T, C), np.float32)
    for core in range(8):
        b, half = core // 2, core % 2
        out[b, half * TPC:(half + 1) * TPC, :] = res.results[core]["outT"].T
    return out
